# revision 8
# baseline (speedup 1.0000x reference)
"""Trainium2 Bass kernel for the 3-layer dilated RNN (nn_DRNN).

Sharding: data-parallel over batch (B=256 -> 32 per core on 8 cores).
Hidden state lives in SBUF rings stored chunk-major transposed:
ring[p, k*1792 + (t%56)*32 + b] = h[t][k*128+p, b], so every matmul
moving operand is a contiguous slice. PSUM tiles are j-major
(col = j*jstride + local), so the batched input projections are single
wide contiguous matmuls (weights stationary, amortized per window) and
a bias matmul doubles as the bank-clearing first write; recurrent
matmuls accumulate on top (dil=1 per step, dil=2 per pair, dil=7 per
septet) and one wide tanh per dependency group on ScalarE writes the
rings. Output projections dump transposed o1/o2/o3 to DRAM; the host
assembles outputs = o1+o2+o3 and the three loss scalars.
"""

import sys

sys.path.insert(0, "/opt/trn_rl_repo")

import numpy as np
import ml_dtypes
import concourse.bass as bass
import concourse.mybir as mybir
from concourse.tile import TileContext
from concourse.bass import ds
from concourse.bass_utils import run_bass_kernel_spmd

F32 = mybir.dt.float32
TANH = mybir.ActivationFunctionType.Tanh

B, S, IN, OUT, H, T = 256, 2048, 64, 64, 512, 7
NCORES = 8
BC = B // NCORES          # 32 batch rows per core
W = 14                    # steps per window (lcm of dilations 2,7)
RING_W = 4
RING_SLOTS = RING_W * W   # 56 steps per ring
RCH = RING_SLOTS * BC     # ring chunk stride (1792 cols)
BODY_W = RING_W

DT = mybir.dt.bfloat16
NPDT = ml_dtypes.bfloat16


def _win_steps(w, nwin, tail):
    return W if w < nwin - 1 or tail == 0 else tail


def _rs(t):
    return (t % RING_SLOTS) * BC


def build_nc(s_total=S):
    nwin_full, tail = divmod(s_total, W)
    nwin = nwin_full + (1 if tail else 0)
    assert nwin >= BODY_W + 3

    nc = bass.Bass()
    xT = nc.dram_tensor("xT", [IN + 1, s_total * BC], DT, kind="ExternalInput")
    wh1t = nc.dram_tensor("wh1t", [128, 2048], DT, kind="ExternalInput")
    wh2t = nc.dram_tensor("wh2t", [128, 2048], DT, kind="ExternalInput")
    wh3t = nc.dram_tensor("wh3t", [128, 2048], DT, kind="ExternalInput")
    wi2t = nc.dram_tensor("wi2t", [128, 2048], DT, kind="ExternalInput")
    wi3t = nc.dram_tensor("wi3t", [128, 2048], DT, kind="ExternalInput")
    wi1taug = nc.dram_tensor("wi1taug", [IN + 1, 512], DT, kind="ExternalInput")
    wot = nc.dram_tensor("wot", [128, 768], DT, kind="ExternalInput")
    cch = nc.dram_tensor("cch", [4, 256], DT, kind="ExternalInput")
    msk = nc.dram_tensor("msk", [4, 512], DT, kind="ExternalInput")  # col//128 mask
    c2f = nc.dram_tensor("c2f", [1, 512], DT, kind="ExternalInput")
    ones1 = nc.dram_tensor("ones1", [1, 512], DT, kind="ExternalInput")
    bov = nc.dram_tensor("bov", [OUT, 3], F32, kind="ExternalInput")
    o1T = nc.dram_tensor("o1T", [OUT, s_total * BC], F32, kind="ExternalOutput")
    o2T = nc.dram_tensor("o2T", [OUT, s_total * BC], F32, kind="ExternalOutput")
    o3T = nc.dram_tensor("o3T", [OUT, s_total * BC], F32, kind="ExternalOutput")
    oTs = [o1T, o2T, o3T]

    with TileContext(nc) as tc:
        import contextlib

        with contextlib.ExitStack() as ctx:
            cpool = ctx.enter_context(tc.tile_pool(name="const", bufs=1))
            rpool = ctx.enter_context(tc.tile_pool(name="rings", bufs=1))
            xpool = ctx.enter_context(tc.tile_pool(name="xw", bufs=2))
            opool = ctx.enter_context(tc.tile_pool(name="ost", bufs=2))
            p1 = ctx.enter_context(tc.tile_pool(name="p1", bufs=1, space="PSUM"))
            p2 = ctx.enter_context(tc.tile_pool(name="p2", bufs=1, space="PSUM"))
            p3 = ctx.enter_context(tc.tile_pool(name="p3", bufs=1, space="PSUM"))
            po = ctx.enter_context(tc.tile_pool(name="po", bufs=1, space="PSUM"))

            wh_sb = {}
            for name, dram in (("wh1", wh1t), ("wh2", wh2t), ("wh3", wh3t),
                               ("wi2", wi2t), ("wi3", wi3t)):
                t = cpool.tile([128, 2048], DT, tag=f"w_{name}")
                nc.sync.dma_start(t[:], dram[:])
                wh_sb[name] = t
            wi1_sb = cpool.tile([IN + 1, 512], DT, tag="w_wi1")
            nc.sync.dma_start(wi1_sb[:], wi1taug[:])
            wot_sb = cpool.tile([128, 768], DT, tag="w_wot")
            nc.sync.dma_start(wot_sb[:], wot[:])
            cch_sb = cpool.tile([4, 256], DT, tag="cch")
            nc.sync.dma_start(cch_sb[:], cch[:])
            msk_sb = cpool.tile([4, 512], DT, tag="msk")
            nc.sync.dma_start(msk_sb[:], msk[:])
            c2f_sb = cpool.tile([1, 512], DT, tag="c2f")
            nc.sync.dma_start(c2f_sb[:], c2f[:])
            ones_sb = cpool.tile([1, 512], DT, tag="ones1")
            nc.sync.dma_start(ones_sb[:], ones1[:])
            bo_sb = cpool.tile([OUT, 3], F32, tag="bo")
            nc.sync.dma_start(bo_sb[:], bov[:])

            rings = {}
            for name in ("h1", "h2", "h3"):
                r = rpool.tile([128, 4 * RCH], DT, tag=f"ring_{name}")
                nc.gpsimd.memset(r[:], 0.0)
                rings[name] = r

            def wst(sb, k, j):
                return sb[:, k * 512 + j * 128:k * 512 + (j + 1) * 128]

            def rmov(ring, t0, nst, k):
                # contiguous [128, nst*32] moving slice of chunk k
                c0 = k * RCH + _rs(t0)
                return ring[:, c0:c0 + nst * BC]

            def rout(ring, t0, nst):
                # [128, 4, nst*32] tanh-out view (4 chunk blocks)
                v = ring[:].rearrange("p (k r) -> p k r", k=4)
                return v[:, :, _rs(t0):_rs(t0) + nst * BC]

            def psjv(ps, jstride, off, n):
                # [128, 4, n] j-blocked psum view
                v = ps[:].rearrange("p (j r) -> p j r", j=4)
                return v[:, :, off:off + n]

            # ---------------- layer 1 ----------------
            def emit_L1(w, xcol):
                nst = _win_steps(w, nwin, tail)
                t0 = w * W
                xw = xpool.tile([IN + 1, W * BC], DT, tag="xw")
                nc.sync.dma_start(xw[:, :nst * BC],
                                  xT[:, xcol] if xcol is not None
                                  else xT[:, t0 * BC: t0 * BC + nst * BC])
                for h0 in (0, 7):
                    hn = min(7, nst - h0)
                    if hn <= 0:
                        continue
                    ps = p1.tile([128, 1024], F32, name="ps1", tag="l1")
                    for j in range(4):
                        nc.tensor.matmul(
                            ps[:, j * 256: j * 256 + hn * BC],
                            wi1_sb[:, j * 128:(j + 1) * 128],
                            xw[:, h0 * BC:(h0 + hn) * BC],
                            start=(j % 2 == 0), stop=True,
                            skip_group_check=True)
                    for s in range(hn):
                        t = t0 + h0 + s
                        for j in range(4):
                            for k in range(4):
                                nc.tensor.matmul(
                                    ps[:, j * 256 + s * BC: j * 256 + (s + 1) * BC],
                                    wst(wh_sb["wh1"], k, j),
                                    rmov(rings["h1"], t - 1, 1, k),
                                    start=False, stop=True, skip_group_check=True)
                        nc.scalar.activation(rout(rings["h1"], t, 1),
                                             psjv(ps, 256, s * BC, BC), TANH)

            # ---------------- layer 2 (dil=2) ----------------
            def emit_L2(w):
                nst = _win_steps(w, nwin, tail)
                t0 = w * W
                ps = p2.tile([128, 2048], F32, name="ps2", tag="l2")
                for j in range(4):
                    nc.tensor.matmul(ps[:, j * 512: j * 512 + nst * BC],
                                     c2f_sb[:, j * 128:(j + 1) * 128],
                                     ones_sb[:, :nst * BC],
                                     start=True, stop=True, skip_group_check=True)
                for j in range(4):
                    for k in range(4):
                        nc.tensor.matmul(ps[:, j * 512: j * 512 + nst * BC],
                                         wst(wh_sb["wi2"], k, j),
                                         rmov(rings["h1"], t0, nst, k),
                                         start=False, stop=True,
                                         skip_group_check=True)
                for p in range(nst // 2):
                    tp = t0 + 2 * p
                    for j in range(4):
                        for k in range(4):
                            nc.tensor.matmul(
                                ps[:, j * 512 + p * 64: j * 512 + (p + 1) * 64],
                                wst(wh_sb["wh2"], k, j),
                                rmov(rings["h2"], tp - 2, 2, k),
                                start=False, stop=True, skip_group_check=True)
                    nc.scalar.activation(rout(rings["h2"], tp, 2),
                                         psjv(ps, 512, p * 64, 64), TANH)

            # ---------------- layer 3 (dil=7) ----------------
            def emit_L3(w):
                nst = _win_steps(w, nwin, tail)
                t0 = w * W
                halves = []
                for g0 in range(0, nst, 7):
                    glen = min(7, nst - g0)
                    halves.append((g0, min(4, glen)))
                    if glen > 4:
                        halves.append((g0 + 4, glen - 4))
                for (h0, hn) in halves:
                    ps = p3.tile([128, 512], F32, name="ps3", tag="l3")
                    nc.tensor.matmul(ps[:], cch_sb[:, 128:256], msk_sb[:],
                                     start=True, stop=True, skip_group_check=True)
                    for j in range(4):
                        for k in range(4):
                            nc.tensor.matmul(
                                ps[:, j * 128: j * 128 + hn * BC],
                                wst(wh_sb["wi3"], k, j),
                                rmov(rings["h2"], t0 + h0, hn, k),
                                start=False, stop=True, skip_group_check=True)
                    for j in range(4):
                        for k in range(4):
                            nc.tensor.matmul(
                                ps[:, j * 128: j * 128 + hn * BC],
                                wst(wh_sb["wh3"], k, j),
                                rmov(rings["h3"], t0 + h0 - 7, hn, k),
                                start=False, stop=True, skip_group_check=True)
                    nc.scalar.activation(rout(rings["h3"], t0 + h0, hn),
                                         psjv(ps, 128, 0, hn * BC), TANH)

            # ---------------- output projections ----------------
            def emit_O(w, ocol):
                nst = _win_steps(w, nwin, tail)
                t0 = w * W
                ncols = nst * BC
                for li, rname in enumerate(("h1", "h2", "h3")):
                    pot = po.tile([OUT, W * BC], F32, name=f"psO{li}", tag="o")
                    for k in range(4):
                        nc.tensor.matmul(pot[:, :ncols],
                                         wot_sb[:, (li * 4 + k) * 64:
                                                (li * 4 + k + 1) * 64],
                                         rmov(rings[rname], t0, nst, k),
                                         start=(k == 0), stop=(k == 3))
                    ot = opool.tile([OUT, W * BC], F32, name=f"ot{li}", tag=f"o{li}")
                    nc.scalar.activation(ot[:, :ncols], pot[:, :ncols], TANH,
                                         bias=bo_sb[:, li:li + 1])
                    if ocol is not None:
                        nc.sync.dma_start(oTs[li][:, ocol], ot[:, :ncols])
                    else:
                        nc.sync.dma_start(oTs[li][:, t0 * BC: t0 * BC + ncols],
                                          ot[:, :ncols])

            def emit_round(w, base=None, wi=0):
                if 0 <= w < nwin:
                    xcol = None if base is None else ds(
                        base + wi * (W * BC), _win_steps(w, nwin, tail) * BC)
                    emit_L1(w, xcol)
                if 0 <= w - 1 < nwin:
                    emit_L2(w - 1)
                if 0 <= w - 2 < nwin:
                    emit_L3(w - 2)
                if 0 <= w - 2 < nwin:
                    ocol = None if base is None else ds(
                        base + (wi - 2) * (W * BC),
                        _win_steps(w - 2, nwin, tail) * BC)
                    emit_O(w - 2, ocol)

            n_loop_groups = nwin_full // BODY_W
            for wi in range(BODY_W):
                emit_round(wi)
            if n_loop_groups > 1:
                stride = BODY_W * W * BC
                with tc.For_i(stride, n_loop_groups * stride, stride,
                              hint_engines=(mybir.EngineType.PE,
                                            mybir.EngineType.Activation)) as base:
                    for wi in range(BODY_W):
                        emit_round(BODY_W + wi, base=base, wi=wi)
            done_w = n_loop_groups * BODY_W
            for w in range(done_w, nwin + 3):
                emit_round(w)

    return nc


def _split_waits(nc, cap=1):
    for bb in nc.m.functions[0].blocks:
        newlist = []
        for ins in bb.instructions:
            si = ins.sync_info
            if si and len(si.on_wait) > cap:
                waits = list(si.on_wait)
                extra, keep = waits[:-cap], waits[-cap:]
                for i, wt in enumerate(extra):
                    nop = mybir.InstNoOp(name=f"{ins.name}-wsplit{i}",
                                         text_hint="wait_split")
                    nop.engine = ins.engine
                    nop.sync_info = mybir.SyncInfo(on_wait=[wt], on_update=[])
                    nc.register_instruction(nop, overwrite=True)
                    newlist.append(nop)
                si.on_wait = keep
                ins.sync_info = si
            newlist.append(ins)
        bb.instructions[:] = newlist


def _prep_weights(inputs):
    f = lambda a: np.asarray(a, np.float32)
    Wh = [f(inputs["Wh1"]), f(inputs["Wh2"]), f(inputs["Wh3"])]
    Wi = [f(inputs["Wi1"]), f(inputs["Wi2"]), f(inputs["Wi3"])]
    Wo = [f(inputs["Wo1"]), f(inputs["Wo2"]), f(inputs["Wo3"])]
    c = [f(inputs["bi1"]) + f(inputs["bh1"]),
         f(inputs["bi2"]) + f(inputs["bh2"]),
         f(inputs["bi3"]) + f(inputs["bh3"])]
    bo = [f(inputs["bo1"]), f(inputs["bo2"]), f(inputs["bo3"])]

    def wt_tiles(Wm):
        out = np.empty((128, 2048), np.float32)
        for k in range(4):
            for j in range(4):
                out[:, k * 512 + j * 128:k * 512 + (j + 1) * 128] = \
                    Wm[j * 128:(j + 1) * 128, k * 128:(k + 1) * 128].T
        return out.astype(NPDT)

    d = {}
    d["wh1t"], d["wh2t"], d["wh3t"] = map(wt_tiles, Wh)
    d["wi2t"], d["wi3t"] = wt_tiles(Wi[1]), wt_tiles(Wi[2])
    wi1aug = np.empty((IN + 1, 512), np.float32)
    wi1aug[:IN] = Wi[0].T
    wi1aug[IN] = c[0]
    d["wi1taug"] = wi1aug.astype(NPDT)
    wot = np.empty((128, 768), np.float32)
    for li in range(3):
        for k in range(4):
            wot[:, (li * 4 + k) * 64:(li * 4 + k + 1) * 64] = \
                Wo[li][:, k * 128:(k + 1) * 128].T
    d["wot"] = wot.astype(NPDT)
    cchunks = np.empty((4, 256), np.float32)
    cchunks[:, :128] = c[1].reshape(4, 128)
    cchunks[:, 128:] = c[2].reshape(4, 128)
    d["cch"] = cchunks.astype(NPDT)
    m = np.zeros((4, 512), np.float32)
    for col in range(512):
        m[col // 128, col] = 1.0
    d["msk"] = m.astype(NPDT)
    d["ones1"] = np.ones((1, 512), np.float32).astype(NPDT)
    d["c2f"] = c[1].reshape(1, 512).astype(NPDT)
    d["bov"] = np.stack(bo, axis=1).copy()
    return d


_CACHED = {}
TRACE = False
LAST_RESULT = None


def kernel(**inputs):
    x = np.asarray(inputs["x"], np.float32)
    s_total = x.shape[1]
    if s_total not in _CACHED:
        nc = build_nc(s_total)
        _split_waits(nc)
        _CACHED[s_total] = nc
    nc = _CACHED[s_total]

    wd = _prep_weights(inputs)
    in_maps = []
    for c in range(NCORES):
        xc = x[c * BC:(c + 1) * BC]
        xt = np.empty((IN + 1, s_total * BC), np.float32)
        xt[:IN] = xc.transpose(2, 1, 0).reshape(IN, s_total * BC)
        xt[IN] = 1.0
        m = dict(wd)
        m["xT"] = xt.astype(NPDT)
        in_maps.append(m)

    global LAST_RESULT
    res = run_bass_kernel_spmd(nc, in_maps, core_ids=list(range(NCORES)),
                               trace=TRACE)
    LAST_RESULT = res

    o1f = np.empty((B, s_total, OUT), np.float32)
    o2f = np.empty((B, s_total, OUT), np.float32)
    o3f = np.empty((B, s_total, OUT), np.float32)
    for c in range(NCORES):
        r = res.results[c]
        for name, dst in (("o1T", o1f), ("o2T", o2f), ("o3T", o3f)):
            a = r[name].reshape(OUT, s_total, BC).transpose(2, 1, 0)
            dst[c * BC:(c + 1) * BC] = a
    outs = o1f + o2f + o3f

    rl = np.float32((o1f.astype(np.float64) ** 2).sum() / (B * OUT))
    dd = o2f[:, T:].astype(np.float64) - o2f[:, :-T].astype(np.float64)
    sl = np.float32((dd ** 2).sum() / (B * OUT))
    o3d = o3f.astype(np.float64)
    s1 = o3d.sum(axis=2)
    s2 = (o3d ** 2).sum(axis=2)
    var = (s2 - s1 ** 2 / OUT) / (OUT - 1)
    ml = np.float32(var.sum() / B)
    return outs, rl, sl, ml


# revision 9
# speedup vs baseline: 1.0027x; 1.0027x over previous
"""Trainium2 Bass kernel for the 3-layer dilated RNN (nn_DRNN).

Sharding: data-parallel over batch (B=256 -> 32 per core on 8 cores).
On-device layout is "hT": hidden state stored transposed as
[128 partitions = h%128, cols = ringslot*128 + (h//128)*32 + b].
PSUM is step-major (col = step_local*128 + j*32 + b), pre-filled with a
bias mask-matmul (which doubles as the bank-clearing first write) plus
the batched input projection (weights stationary, moving = previous
layer's hT window); the recurrent matmuls accumulate on top (dil=1 ->
per step, dil=2 -> pairs, dil=7 -> septet halves) and one wide tanh per
dependency group on ScalarE writes the hT ring in SBUF. Output
projections produce transposed o1/o2/o3 streams dumped to DRAM; the
host assembles outputs = o1+o2+o3 and the three loss scalars.
"""

import sys

sys.path.insert(0, "/opt/trn_rl_repo")

import numpy as np
import ml_dtypes
import concourse.bass as bass
import concourse.mybir as mybir
from concourse.tile import TileContext
from concourse.bass import ds
from concourse.bass_utils import run_bass_kernel_spmd

F32 = mybir.dt.float32
TANH = mybir.ActivationFunctionType.Tanh

B, S, IN, OUT, H, T = 256, 2048, 64, 64, 512, 7
NCORES = 8
BC = B // NCORES          # 32 batch rows per core
W = 14                    # steps per window (lcm of dilations 2,7)
RING_W = 4                # ring capacity in windows
RING_SLOTS = RING_W * W   # 56 steps
BODY_W = RING_W           # windows per loop body (ring offsets static)

DT = mybir.dt.bfloat16
NPDT = ml_dtypes.bfloat16


def _win_steps(w, nwin, tail):
    return W if w < nwin - 1 or tail == 0 else tail


def _rbase(t):
    return (t % RING_SLOTS) * 128


def build_nc(s_total=S):
    nwin_full, tail = divmod(s_total, W)
    nwin = nwin_full + (1 if tail else 0)
    assert nwin >= BODY_W + 3

    nc = bass.Bass()
    xT = nc.dram_tensor("xT", [IN + 1, s_total * BC], DT, kind="ExternalInput")
    wh1t = nc.dram_tensor("wh1t", [128, 2048], DT, kind="ExternalInput")
    wh2t = nc.dram_tensor("wh2t", [128, 2048], DT, kind="ExternalInput")
    wh3t = nc.dram_tensor("wh3t", [128, 2048], DT, kind="ExternalInput")
    wi2t = nc.dram_tensor("wi2t", [128, 2048], DT, kind="ExternalInput")
    wi3t = nc.dram_tensor("wi3t", [128, 2048], DT, kind="ExternalInput")
    wi1taug = nc.dram_tensor("wi1taug", [IN + 1, 512], DT, kind="ExternalInput")
    wot = nc.dram_tensor("wot", [128, 768], DT, kind="ExternalInput")
    cch = nc.dram_tensor("cch", [4, 256], DT, kind="ExternalInput")   # c2|c3 chunks
    msk = nc.dram_tensor("msk", [4, 512], DT, kind="ExternalInput")   # j-block 0/1 mask
    bov = nc.dram_tensor("bov", [OUT, 3], F32, kind="ExternalInput")
    o1T = nc.dram_tensor("o1T", [OUT, s_total * BC], F32, kind="ExternalOutput")
    o2T = nc.dram_tensor("o2T", [OUT, s_total * BC], F32, kind="ExternalOutput")
    o3T = nc.dram_tensor("o3T", [OUT, s_total * BC], F32, kind="ExternalOutput")
    oTs = [o1T, o2T, o3T]

    with TileContext(nc) as tc:
        import contextlib

        with contextlib.ExitStack() as ctx:
            cpool = ctx.enter_context(tc.tile_pool(name="const", bufs=1))
            rpool = ctx.enter_context(tc.tile_pool(name="rings", bufs=1))
            xpool = ctx.enter_context(tc.tile_pool(name="xw", bufs=2))
            opool = ctx.enter_context(tc.tile_pool(name="ost", bufs=2))
            p1 = ctx.enter_context(tc.tile_pool(name="p1", bufs=1, space="PSUM"))
            p2 = ctx.enter_context(tc.tile_pool(name="p2", bufs=1, space="PSUM"))
            p3 = ctx.enter_context(tc.tile_pool(name="p3", bufs=1, space="PSUM"))
            po = ctx.enter_context(tc.tile_pool(name="po", bufs=2, space="PSUM"))

            wh_sb = {}
            for name, dram in (("wh1", wh1t), ("wh2", wh2t), ("wh3", wh3t),
                               ("wi2", wi2t), ("wi3", wi3t)):
                t = cpool.tile([128, 2048], DT, tag=f"w_{name}")
                nc.sync.dma_start(t[:], dram[:])
                wh_sb[name] = t
            wi1_sb = cpool.tile([IN + 1, 512], DT, tag="w_wi1")
            nc.sync.dma_start(wi1_sb[:], wi1taug[:])
            wot_sb = cpool.tile([128, 768], DT, tag="w_wot")
            nc.sync.dma_start(wot_sb[:], wot[:])
            cch_sb = cpool.tile([4, 256], DT, tag="cch")
            nc.sync.dma_start(cch_sb[:], cch[:])
            msk_sb = cpool.tile([4, 512], DT, tag="msk")
            nc.sync.dma_start(msk_sb[:], msk[:])
            bo_sb = cpool.tile([OUT, 3], F32, tag="bo")
            nc.sync.dma_start(bo_sb[:], bov[:])

            rings = {}
            for name in ("h1", "h2", "h3"):
                r = rpool.tile([128, RING_SLOTS * 128], DT, tag=f"ring_{name}")
                nc.gpsimd.memset(r[:], 0.0)
                rings[name] = r

            def wst(sb, k, j):
                return sb[:, k * 512 + j * 128:k * 512 + (j + 1) * 128]

            def ring_mov(ring, t0, nst, k):
                base = _rbase(t0)
                v = ring[:, base:base + nst * 128]
                v = v.rearrange("p (s c) -> p s c", c=128)
                return v[:, :, k * 32:(k + 1) * 32]

            def ps_j(tile, coff, nst, j):
                # [128, nst, 32] strided view: cols coff + s*128 + j*32
                v = tile[:, coff:coff + nst * 128]
                v = v.rearrange("p (s c) -> p s c", c=128)
                return v[:, :, j * 32:(j + 1) * 32]

            # ---------------- layer 1 ----------------
            def emit_L1(w, xcol):
                nst = _win_steps(w, nwin, tail)
                t0 = w * W
                xw = xpool.tile([IN + 1, W * BC], DT, tag="xw")
                nc.sync.dma_start(xw[:, :nst * BC],
                                  xT[:, xcol] if xcol is not None
                                  else xT[:, t0 * BC: t0 * BC + nst * BC])
                for g0 in range(0, nst, 4):
                    gn = min(4, nst - g0)
                    ps = p1.tile([128, 512], F32, name="ps1", tag="l1")
                    xv = xw[:, g0 * 32:(g0 + gn) * 32]
                    xv = xv.rearrange("p (s c) -> p s c", c=32)
                    for j in range(4):
                        nc.tensor.matmul(ps_j(ps, 0, gn, j),
                                         wi1_sb[:, j * 128:(j + 1) * 128],
                                         xv, start=(j == 0), stop=True,
                                         skip_group_check=True)
                    for s in range(gn):
                        t = t0 + g0 + s
                        for j in range(4):
                            for k in range(4):
                                nc.tensor.matmul(
                                    ps[:, s * 128 + j * 32: s * 128 + j * 32 + 32],
                                    wst(wh_sb["wh1"], k, j),
                                    rings["h1"][:, _rbase(t - 1) + k * 32:
                                                _rbase(t - 1) + k * 32 + 32],
                                    start=False, stop=True, skip_group_check=True)
                        nc.scalar.activation(
                            rings["h1"][:, _rbase(t):_rbase(t) + 128],
                            ps[:, s * 128:(s + 1) * 128], TANH)

            # ---------------- layers 2 (dil=2, window-wide psum) ----------
            def emit_L2(w):
                nst = _win_steps(w, nwin, tail)
                t0 = w * W
                nbank = (nst * 128 + 511) // 512
                ps = p2.tile([128, 2048], F32, name="ps2", tag="l2")
                for b in range(nbank):
                    s0 = b * 4
                    sn = min(4, nst - s0)
                    nc.tensor.matmul(ps[:, b * 512:b * 512 + sn * 128],
                                     cch_sb[:, 0:128], msk_sb[:, :sn * 128],
                                     start=True, stop=True, skip_group_check=True)
                for j in range(4):
                    for k in range(4):
                        for b in range(nbank):
                            s0 = b * 4
                            sn = min(4, nst - s0)
                            nc.tensor.matmul(
                                ps_j(ps, b * 512, sn, j),
                                wst(wh_sb["wi2"], k, j),
                                ring_mov(rings["h1"], t0 + s0, sn, k),
                                start=False, stop=True, skip_group_check=True)
                for p in range(nst // 2):
                    tp = t0 + 2 * p
                    for j in range(4):
                        for k in range(4):
                            nc.tensor.matmul(
                                ps_j(ps, p * 256, 2, j),
                                wst(wh_sb["wh2"], k, j),
                                ring_mov(rings["h2"], tp - 2, 2, k),
                                start=False, stop=True, skip_group_check=True)
                    nc.scalar.activation(
                        rings["h2"][:, _rbase(tp):_rbase(tp) + 256],
                        ps[:, p * 256:(p + 1) * 256], TANH)

            # ---------------- layer 3 (dil=7, half-septet psum) ----------
            def emit_L3(w):
                nst = _win_steps(w, nwin, tail)
                t0 = w * W
                halves = []
                for g0 in range(0, nst, 7):
                    glen = min(7, nst - g0)
                    halves.append((g0, min(4, glen)))
                    if glen > 4:
                        halves.append((g0 + 4, glen - 4))
                for (h0, hn) in halves:
                    ps = p3.tile([128, 512], F32, name="ps3", tag="l3")
                    nc.tensor.matmul(ps[:, :hn * 128],
                                     cch_sb[:, 128:256], msk_sb[:, :hn * 128],
                                     start=True, stop=True, skip_group_check=True)
                    for j in range(4):
                        for k in range(4):
                            nc.tensor.matmul(
                                ps_j(ps, 0, hn, j),
                                wst(wh_sb["wi3"], k, j),
                                ring_mov(rings["h2"], t0 + h0, hn, k),
                                start=False, stop=True, skip_group_check=True)
                    for j in range(4):
                        for k in range(4):
                            nc.tensor.matmul(
                                ps_j(ps, 0, hn, j),
                                wst(wh_sb["wh3"], k, j),
                                ring_mov(rings["h3"], t0 + h0 - 7, hn, k),
                                start=False, stop=True, skip_group_check=True)
                    nc.scalar.activation(
                        rings["h3"][:, _rbase(t0 + h0):_rbase(t0 + h0) + hn * 128],
                        ps[:, :hn * 128], TANH)

            # ---------------- output projections ----------------
            def emit_O(w, ocol):
                nst = _win_steps(w, nwin, tail)
                t0 = w * W
                ncols = nst * BC
                for li, rname in enumerate(("h1", "h2", "h3")):
                    pot = po.tile([OUT, W * BC], F32, name=f"psO{li}", tag="o")
                    for k in range(4):
                        nc.tensor.matmul(pot[:, :ncols],
                                         wot_sb[:, (li * 4 + k) * 64:
                                                (li * 4 + k + 1) * 64],
                                         ring_mov(rings[rname], t0, nst, k),
                                         start=(k == 0), stop=(k == 3))
                    ot = opool.tile([OUT, W * BC], F32, name=f"ot{li}", tag=f"o{li}")
                    nc.scalar.activation(ot[:, :ncols], pot[:, :ncols], TANH,
                                         bias=bo_sb[:, li:li + 1])
                    if ocol is not None:
                        nc.sync.dma_start(oTs[li][:, ocol], ot[:, :ncols])
                    else:
                        nc.sync.dma_start(oTs[li][:, t0 * BC: t0 * BC + ncols],
                                          ot[:, :ncols])

            def emit_round(w, base=None, wi=0):
                if 0 <= w < nwin:
                    xcol = None if base is None else ds(
                        base + wi * (W * BC), _win_steps(w, nwin, tail) * BC)
                    emit_L1(w, xcol)
                if 0 <= w - 1 < nwin:
                    emit_L2(w - 1)
                if 0 <= w - 2 < nwin:
                    emit_L3(w - 2)
                if 0 <= w - 2 < nwin:
                    ocol = None if base is None else ds(
                        base + (wi - 2) * (W * BC),
                        _win_steps(w - 2, nwin, tail) * BC)
                    emit_O(w - 2, ocol)

            n_loop_groups = nwin_full // BODY_W
            for wi in range(BODY_W):
                emit_round(wi)
            if n_loop_groups > 1:
                stride = BODY_W * W * BC
                with tc.For_i(stride, n_loop_groups * stride, stride,
                              hint_engines=(mybir.EngineType.PE,
                                            mybir.EngineType.Activation)) as base:
                    for wi in range(BODY_W):
                        emit_round(BODY_W + wi, base=base, wi=wi)
            done_w = n_loop_groups * BODY_W
            for w in range(done_w, nwin + 3):
                emit_round(w)

    return nc


def _split_waits(nc, cap=1):
    for bb in nc.m.functions[0].blocks:
        newlist = []
        for ins in bb.instructions:
            si = ins.sync_info
            if si and len(si.on_wait) > cap:
                waits = list(si.on_wait)
                extra, keep = waits[:-cap], waits[-cap:]
                for i, wt in enumerate(extra):
                    nop = mybir.InstNoOp(name=f"{ins.name}-wsplit{i}",
                                         text_hint="wait_split")
                    nop.engine = ins.engine
                    nop.sync_info = mybir.SyncInfo(on_wait=[wt], on_update=[])
                    nc.register_instruction(nop, overwrite=True)
                    newlist.append(nop)
                si.on_wait = keep
                ins.sync_info = si
            newlist.append(ins)
        bb.instructions[:] = newlist


def _prep_weights(inputs):
    f = lambda a: np.asarray(a, np.float32)
    Wh = [f(inputs["Wh1"]), f(inputs["Wh2"]), f(inputs["Wh3"])]
    Wi = [f(inputs["Wi1"]), f(inputs["Wi2"]), f(inputs["Wi3"])]
    Wo = [f(inputs["Wo1"]), f(inputs["Wo2"]), f(inputs["Wo3"])]
    c = [f(inputs["bi1"]) + f(inputs["bh1"]),
         f(inputs["bi2"]) + f(inputs["bh2"]),
         f(inputs["bi3"]) + f(inputs["bh3"])]
    bo = [f(inputs["bo1"]), f(inputs["bo2"]), f(inputs["bo3"])]

    def wt_tiles(Wm):
        out = np.empty((128, 2048), np.float32)
        for k in range(4):
            for j in range(4):
                out[:, k * 512 + j * 128:k * 512 + (j + 1) * 128] = \
                    Wm[j * 128:(j + 1) * 128, k * 128:(k + 1) * 128].T
        return out.astype(NPDT)

    d = {}
    d["wh1t"], d["wh2t"], d["wh3t"] = map(wt_tiles, Wh)
    d["wi2t"], d["wi3t"] = wt_tiles(Wi[1]), wt_tiles(Wi[2])
    wi1aug = np.empty((IN + 1, 512), np.float32)
    wi1aug[:IN] = Wi[0].T
    wi1aug[IN] = c[0]
    d["wi1taug"] = wi1aug.astype(NPDT)
    wot = np.empty((128, 768), np.float32)
    for li in range(3):
        for k in range(4):
            wot[:, (li * 4 + k) * 64:(li * 4 + k + 1) * 64] = \
                Wo[li][:, k * 128:(k + 1) * 128].T
    d["wot"] = wot.astype(NPDT)
    # c2/c3 chunk rows [4, 128] stacked -> [4, 256]
    cchunks = np.empty((4, 256), np.float32)
    cchunks[:, :128] = c[1].reshape(4, 128)
    cchunks[:, 128:] = c[2].reshape(4, 128)
    d["cch"] = cchunks.astype(NPDT)
    # mask[j', s*128 + j*32 + b] = (j == j')
    m = np.zeros((4, 512), np.float32)
    for col in range(512):
        m[(col // 32) % 4, col] = 1.0
    d["msk"] = m.astype(NPDT)
    d["bov"] = np.stack(bo, axis=1).copy()
    return d


_CACHED = {}
TRACE = False
LAST_RESULT = None


def kernel(**inputs):
    x = np.asarray(inputs["x"], np.float32)
    s_total = x.shape[1]
    if s_total not in _CACHED:
        nc = build_nc(s_total)
        _split_waits(nc)
        _CACHED[s_total] = nc
    nc = _CACHED[s_total]

    wd = _prep_weights(inputs)
    in_maps = []
    for c in range(NCORES):
        xc = x[c * BC:(c + 1) * BC]
        xt = np.empty((IN + 1, s_total * BC), np.float32)
        xt[:IN] = xc.transpose(2, 1, 0).reshape(IN, s_total * BC)
        xt[IN] = 1.0
        m = dict(wd)
        m["xT"] = xt.astype(NPDT)
        in_maps.append(m)

    global LAST_RESULT
    res = run_bass_kernel_spmd(nc, in_maps, core_ids=list(range(NCORES)),
                               trace=TRACE)
    LAST_RESULT = res

    o1f = np.empty((B, s_total, OUT), np.float32)
    o2f = np.empty((B, s_total, OUT), np.float32)
    o3f = np.empty((B, s_total, OUT), np.float32)
    for c in range(NCORES):
        r = res.results[c]
        for name, dst in (("o1T", o1f), ("o2T", o2f), ("o3T", o3f)):
            a = r[name].reshape(OUT, s_total, BC).transpose(2, 1, 0)
            dst[c * BC:(c + 1) * BC] = a
    outs = o1f + o2f + o3f

    rl = np.float32((o1f.astype(np.float64) ** 2).sum() / (B * OUT))
    dd = o2f[:, T:].astype(np.float64) - o2f[:, :-T].astype(np.float64)
    sl = np.float32((dd ** 2).sum() / (B * OUT))
    o3d = o3f.astype(np.float64)
    s1 = o3d.sum(axis=2)
    s2 = (o3d ** 2).sum(axis=2)
    var = (s2 - s1 ** 2 / OUT) / (OUT - 1)
    ml = np.float32(var.sum() / B)
    return outs, rl, sl, ml


# revision 10
# speedup vs baseline: 1.0166x; 1.0138x over previous
"""Trainium2 Bass kernel for the 3-layer dilated RNN (nn_DRNN).

Sharding: data-parallel over batch (B=256 -> 32 per core on 8 cores).
On-device layout is "hT": hidden state stored transposed as
[128 partitions = h%128, cols = ringslot*128 + (h//128)*32 + b].
PSUM is step-major (col = step_local*128 + j*32 + b), pre-filled with a
bias mask-matmul (which doubles as the bank-clearing first write) plus
the batched input projection (weights stationary, moving = previous
layer's hT window); the recurrent matmuls accumulate on top (dil=1 ->
per step, dil=2 -> pairs, dil=7 -> septet halves) and one wide tanh per
dependency group on ScalarE writes the hT ring in SBUF. Output
projections produce transposed o1/o2/o3 streams dumped to DRAM; the
host assembles outputs = o1+o2+o3 and the three loss scalars.
"""

import sys

sys.path.insert(0, "/opt/trn_rl_repo")

import numpy as np
import ml_dtypes
import concourse.bass as bass
import concourse.mybir as mybir
from concourse.tile import TileContext
from concourse.bass import ds
from concourse.bass_utils import run_bass_kernel_spmd

F32 = mybir.dt.float32
TANH = mybir.ActivationFunctionType.Tanh

B, S, IN, OUT, H, T = 256, 2048, 64, 64, 512, 7
NCORES = 8
BC = B // NCORES          # 32 batch rows per core
W = 14                    # steps per window (lcm of dilations 2,7)
RING_W = 4                # ring capacity in windows
RING_SLOTS = RING_W * W   # 56 steps
BODY_W = RING_W           # windows per loop body (ring offsets static)

DT = mybir.dt.bfloat16
NPDT = ml_dtypes.bfloat16


def _win_steps(w, nwin, tail):
    return W if w < nwin - 1 or tail == 0 else tail


def _rbase(t):
    return (t % RING_SLOTS) * 128


def build_nc(s_total=S):
    nwin_full, tail = divmod(s_total, W)
    nwin = nwin_full + (1 if tail else 0)
    assert nwin >= BODY_W + 3

    nc = bass.Bass()
    xT = nc.dram_tensor("xT", [IN + 1, s_total * BC], DT, kind="ExternalInput")
    wh1t = nc.dram_tensor("wh1t", [128, 2048], DT, kind="ExternalInput")
    wh2t = nc.dram_tensor("wh2t", [128, 2048], DT, kind="ExternalInput")
    wh3t = nc.dram_tensor("wh3t", [128, 2048], DT, kind="ExternalInput")
    wi2t = nc.dram_tensor("wi2t", [128, 2048], DT, kind="ExternalInput")
    wi3t = nc.dram_tensor("wi3t", [128, 2048], DT, kind="ExternalInput")
    wi1taug = nc.dram_tensor("wi1taug", [IN + 1, 512], DT, kind="ExternalInput")
    wot = nc.dram_tensor("wot", [128, 768], DT, kind="ExternalInput")
    cch = nc.dram_tensor("cch", [4, 256], DT, kind="ExternalInput")   # c2|c3 chunks
    msk = nc.dram_tensor("msk", [4, 512], DT, kind="ExternalInput")   # j-block 0/1 mask
    bov = nc.dram_tensor("bov", [OUT, 3], F32, kind="ExternalInput")
    o1T = nc.dram_tensor("o1T", [OUT, s_total * BC], F32, kind="ExternalOutput")
    o2T = nc.dram_tensor("o2T", [OUT, s_total * BC], F32, kind="ExternalOutput")
    o3T = nc.dram_tensor("o3T", [OUT, s_total * BC], F32, kind="ExternalOutput")
    oTs = [o1T, o2T, o3T]

    with TileContext(nc) as tc:
        import contextlib

        with contextlib.ExitStack() as ctx:
            cpool = ctx.enter_context(tc.tile_pool(name="const", bufs=1))
            rpool = ctx.enter_context(tc.tile_pool(name="rings", bufs=1))
            xpool = ctx.enter_context(tc.tile_pool(name="xw", bufs=2))
            opool = ctx.enter_context(tc.tile_pool(name="ost", bufs=2))
            p1 = ctx.enter_context(tc.tile_pool(name="p1", bufs=1, space="PSUM"))
            p2 = ctx.enter_context(tc.tile_pool(name="p2", bufs=1, space="PSUM"))
            p3 = ctx.enter_context(tc.tile_pool(name="p3", bufs=1, space="PSUM"))
            po = ctx.enter_context(tc.tile_pool(name="po", bufs=2, space="PSUM"))

            wh_sb = {}
            for name, dram in (("wh1", wh1t), ("wh2", wh2t), ("wh3", wh3t),
                               ("wi2", wi2t), ("wi3", wi3t)):
                t = cpool.tile([128, 2048], DT, tag=f"w_{name}")
                nc.sync.dma_start(t[:], dram[:])
                wh_sb[name] = t
            wi1_sb = cpool.tile([IN + 1, 512], DT, tag="w_wi1")
            nc.sync.dma_start(wi1_sb[:], wi1taug[:])
            wot_sb = cpool.tile([128, 768], DT, tag="w_wot")
            nc.sync.dma_start(wot_sb[:], wot[:])
            cch_sb = cpool.tile([4, 256], DT, tag="cch")
            nc.sync.dma_start(cch_sb[:], cch[:])
            msk_sb = cpool.tile([4, 512], DT, tag="msk")
            nc.sync.dma_start(msk_sb[:], msk[:])
            bo_sb = cpool.tile([OUT, 3], F32, tag="bo")
            nc.sync.dma_start(bo_sb[:], bov[:])

            rings = {}
            for name in ("h1", "h2", "h3"):
                r = rpool.tile([128, RING_SLOTS * 128], DT, tag=f"ring_{name}")
                nc.gpsimd.memset(r[:], 0.0)
                rings[name] = r

            def wst(sb, k, j):
                return sb[:, k * 512 + j * 128:k * 512 + (j + 1) * 128]

            def ring_mov(ring, t0, nst, k):
                base = _rbase(t0)
                v = ring[:, base:base + nst * 128]
                v = v.rearrange("p (s c) -> p s c", c=128)
                return v[:, :, k * 32:(k + 1) * 32]

            def ps_j(tile, coff, nst, j):
                # [128, nst, 32] strided view: cols coff + s*128 + j*32
                v = tile[:, coff:coff + nst * 128]
                v = v.rearrange("p (s c) -> p s c", c=128)
                return v[:, :, j * 32:(j + 1) * 32]

            # ---------------- layer 1 ----------------
            def emit_L1(w, xcol):
                nst = _win_steps(w, nwin, tail)
                t0 = w * W
                xw = xpool.tile([IN + 1, W * BC], DT, tag="xw")
                nc.sync.dma_start(xw[:, :nst * BC],
                                  xT[:, xcol] if xcol is not None
                                  else xT[:, t0 * BC: t0 * BC + nst * BC])
                for g0 in range(0, nst, 4):
                    gn = min(4, nst - g0)
                    ps = p1.tile([128, 512], F32, name="ps1", tag="l1")
                    xv = xw[:, g0 * 32:(g0 + gn) * 32]
                    xv = xv.rearrange("p (s c) -> p s c", c=32)
                    for j in range(4):
                        nc.tensor.matmul(ps_j(ps, 0, gn, j),
                                         wi1_sb[:, j * 128:(j + 1) * 128],
                                         xv, start=(j == 0), stop=True,
                                         skip_group_check=True)
                    for s in range(gn):
                        t = t0 + g0 + s
                        for j in range(4):
                            for k in range(4):
                                nc.tensor.matmul(
                                    ps[:, s * 128 + j * 32: s * 128 + j * 32 + 32],
                                    wst(wh_sb["wh1"], k, j),
                                    rings["h1"][:, _rbase(t - 1) + k * 32:
                                                _rbase(t - 1) + k * 32 + 32],
                                    start=False, stop=True, skip_group_check=True)
                        nc.scalar.activation(
                            rings["h1"][:, _rbase(t):_rbase(t) + 128],
                            ps[:, s * 128:(s + 1) * 128], TANH)

            # ---------------- layers 2 (dil=2, window-wide psum) ----------
            def emit_L2(w):
                nst = _win_steps(w, nwin, tail)
                t0 = w * W
                nbank = (nst * 128 + 511) // 512
                ps = p2.tile([128, 2048], F32, name="ps2", tag="l2")
                for b in range(nbank):
                    s0 = b * 4
                    sn = min(4, nst - s0)
                    nc.tensor.matmul(ps[:, b * 512:b * 512 + sn * 128],
                                     cch_sb[:, 0:128], msk_sb[:, :sn * 128],
                                     start=True, stop=True, skip_group_check=True)
                for j in range(4):
                    for k in range(4):
                        for b in range(nbank):
                            s0 = b * 4
                            sn = min(4, nst - s0)
                            nc.tensor.matmul(
                                ps_j(ps, b * 512, sn, j),
                                wst(wh_sb["wi2"], k, j),
                                ring_mov(rings["h1"], t0 + s0, sn, k),
                                start=False, stop=True, skip_group_check=True)
                for p in range(nst // 2):
                    tp = t0 + 2 * p
                    for j in range(4):
                        for k in range(4):
                            nc.tensor.matmul(
                                ps_j(ps, p * 256, 2, j),
                                wst(wh_sb["wh2"], k, j),
                                ring_mov(rings["h2"], tp - 2, 2, k),
                                start=False, stop=True, skip_group_check=True)
                    nc.scalar.activation(
                        rings["h2"][:, _rbase(tp):_rbase(tp) + 256],
                        ps[:, p * 256:(p + 1) * 256], TANH)

            # ---------------- layer 3 (dil=7, half-septet psum) ----------
            def emit_L3(w):
                nst = _win_steps(w, nwin, tail)
                t0 = w * W
                halves = []
                for g0 in range(0, nst, 7):
                    glen = min(7, nst - g0)
                    halves.append((g0, min(4, glen)))
                    if glen > 4:
                        halves.append((g0 + 4, glen - 4))
                for (h0, hn) in halves:
                    ps = p3.tile([128, 512], F32, name="ps3", tag="l3")
                    nc.tensor.matmul(ps[:, :hn * 128],
                                     cch_sb[:, 128:256], msk_sb[:, :hn * 128],
                                     start=True, stop=True, skip_group_check=True)
                    for j in range(4):
                        for k in range(4):
                            nc.tensor.matmul(
                                ps_j(ps, 0, hn, j),
                                wst(wh_sb["wi3"], k, j),
                                ring_mov(rings["h2"], t0 + h0, hn, k),
                                start=False, stop=True, skip_group_check=True)
                    for j in range(4):
                        for k in range(4):
                            nc.tensor.matmul(
                                ps_j(ps, 0, hn, j),
                                wst(wh_sb["wh3"], k, j),
                                ring_mov(rings["h3"], t0 + h0 - 7, hn, k),
                                start=False, stop=True, skip_group_check=True)
                    nc.scalar.activation(
                        rings["h3"][:, _rbase(t0 + h0):_rbase(t0 + h0) + hn * 128],
                        ps[:, :hn * 128], TANH)

            # ---------------- output projections ----------------
            def emit_O(w, ocol):
                nst = _win_steps(w, nwin, tail)
                t0 = w * W
                ncols = nst * BC
                for li, rname in enumerate(("h1", "h2", "h3")):
                    pot = po.tile([OUT, W * BC], F32, name=f"psO{li}", tag="o")
                    for k in range(4):
                        nc.tensor.matmul(pot[:, :ncols],
                                         wot_sb[:, (li * 4 + k) * 64:
                                                (li * 4 + k + 1) * 64],
                                         ring_mov(rings[rname], t0, nst, k),
                                         start=(k == 0), stop=(k == 3))
                    ot = opool.tile([OUT, W * BC], F32, name=f"ot{li}", tag=f"o{li}")
                    nc.scalar.activation(ot[:, :ncols], pot[:, :ncols], TANH,
                                         bias=bo_sb[:, li:li + 1])
                    if ocol is not None:
                        nc.sync.dma_start(oTs[li][:, ocol], ot[:, :ncols])
                    else:
                        nc.sync.dma_start(oTs[li][:, t0 * BC: t0 * BC + ncols],
                                          ot[:, :ncols])

            def emit_round(w, base=None, wi=0):
                if 0 <= w < nwin:
                    xcol = None if base is None else ds(
                        base + wi * (W * BC), _win_steps(w, nwin, tail) * BC)
                    emit_L1(w, xcol)
                if 0 <= w - 1 < nwin:
                    emit_L2(w - 1)
                if 0 <= w - 2 < nwin:
                    emit_L3(w - 2)
                if 0 <= w - 2 < nwin:
                    ocol = None if base is None else ds(
                        base + (wi - 2) * (W * BC),
                        _win_steps(w - 2, nwin, tail) * BC)
                    emit_O(w - 2, ocol)

            n_loop_groups = nwin_full // BODY_W
            for wi in range(BODY_W):
                emit_round(wi)
            if n_loop_groups > 1:
                stride = BODY_W * W * BC
                with tc.For_i(stride, n_loop_groups * stride, stride,
                              staggered_reset=True,
                              hint_engines=(mybir.EngineType.PE,
                                            mybir.EngineType.Activation)) as base:
                    for wi in range(BODY_W):
                        emit_round(BODY_W + wi, base=base, wi=wi)
            done_w = n_loop_groups * BODY_W
            for w in range(done_w, nwin + 3):
                emit_round(w)

    return nc


def _split_waits(nc, cap=1):
    for bb in nc.m.functions[0].blocks:
        newlist = []
        for ins in bb.instructions:
            si = ins.sync_info
            if si and len(si.on_wait) > cap:
                waits = list(si.on_wait)
                extra, keep = waits[:-cap], waits[-cap:]
                for i, wt in enumerate(extra):
                    nop = mybir.InstNoOp(name=f"{ins.name}-wsplit{i}",
                                         text_hint="wait_split")
                    nop.engine = ins.engine
                    nop.sync_info = mybir.SyncInfo(on_wait=[wt], on_update=[])
                    nc.register_instruction(nop, overwrite=True)
                    newlist.append(nop)
                si.on_wait = keep
                ins.sync_info = si
            newlist.append(ins)
        bb.instructions[:] = newlist


def _prep_weights(inputs):
    f = lambda a: np.asarray(a, np.float32)
    Wh = [f(inputs["Wh1"]), f(inputs["Wh2"]), f(inputs["Wh3"])]
    Wi = [f(inputs["Wi1"]), f(inputs["Wi2"]), f(inputs["Wi3"])]
    Wo = [f(inputs["Wo1"]), f(inputs["Wo2"]), f(inputs["Wo3"])]
    c = [f(inputs["bi1"]) + f(inputs["bh1"]),
         f(inputs["bi2"]) + f(inputs["bh2"]),
         f(inputs["bi3"]) + f(inputs["bh3"])]
    bo = [f(inputs["bo1"]), f(inputs["bo2"]), f(inputs["bo3"])]

    def wt_tiles(Wm):
        out = np.empty((128, 2048), np.float32)
        for k in range(4):
            for j in range(4):
                out[:, k * 512 + j * 128:k * 512 + (j + 1) * 128] = \
                    Wm[j * 128:(j + 1) * 128, k * 128:(k + 1) * 128].T
        return out.astype(NPDT)

    d = {}
    d["wh1t"], d["wh2t"], d["wh3t"] = map(wt_tiles, Wh)
    d["wi2t"], d["wi3t"] = wt_tiles(Wi[1]), wt_tiles(Wi[2])
    wi1aug = np.empty((IN + 1, 512), np.float32)
    wi1aug[:IN] = Wi[0].T
    wi1aug[IN] = c[0]
    d["wi1taug"] = wi1aug.astype(NPDT)
    wot = np.empty((128, 768), np.float32)
    for li in range(3):
        for k in range(4):
            wot[:, (li * 4 + k) * 64:(li * 4 + k + 1) * 64] = \
                Wo[li][:, k * 128:(k + 1) * 128].T
    d["wot"] = wot.astype(NPDT)
    # c2/c3 chunk rows [4, 128] stacked -> [4, 256]
    cchunks = np.empty((4, 256), np.float32)
    cchunks[:, :128] = c[1].reshape(4, 128)
    cchunks[:, 128:] = c[2].reshape(4, 128)
    d["cch"] = cchunks.astype(NPDT)
    # mask[j', s*128 + j*32 + b] = (j == j')
    m = np.zeros((4, 512), np.float32)
    for col in range(512):
        m[(col // 32) % 4, col] = 1.0
    d["msk"] = m.astype(NPDT)
    d["bov"] = np.stack(bo, axis=1).copy()
    return d


_CACHED = {}
TRACE = False
LAST_RESULT = None


def kernel(**inputs):
    x = np.asarray(inputs["x"], np.float32)
    s_total = x.shape[1]
    if s_total not in _CACHED:
        nc = build_nc(s_total)
        _split_waits(nc)
        _CACHED[s_total] = nc
    nc = _CACHED[s_total]

    wd = _prep_weights(inputs)
    in_maps = []
    for c in range(NCORES):
        xc = x[c * BC:(c + 1) * BC]
        xt = np.empty((IN + 1, s_total * BC), np.float32)
        xt[:IN] = xc.transpose(2, 1, 0).reshape(IN, s_total * BC)
        xt[IN] = 1.0
        m = dict(wd)
        m["xT"] = xt.astype(NPDT)
        in_maps.append(m)

    global LAST_RESULT
    res = run_bass_kernel_spmd(nc, in_maps, core_ids=list(range(NCORES)),
                               trace=TRACE)
    LAST_RESULT = res

    o1f = np.empty((B, s_total, OUT), np.float32)
    o2f = np.empty((B, s_total, OUT), np.float32)
    o3f = np.empty((B, s_total, OUT), np.float32)
    for c in range(NCORES):
        r = res.results[c]
        for name, dst in (("o1T", o1f), ("o2T", o2f), ("o3T", o3f)):
            a = r[name].reshape(OUT, s_total, BC).transpose(2, 1, 0)
            dst[c * BC:(c + 1) * BC] = a
    outs = o1f + o2f + o3f

    rl = np.float32((o1f.astype(np.float64) ** 2).sum() / (B * OUT))
    dd = o2f[:, T:].astype(np.float64) - o2f[:, :-T].astype(np.float64)
    sl = np.float32((dd ** 2).sum() / (B * OUT))
    o3d = o3f.astype(np.float64)
    s1 = o3d.sum(axis=2)
    s2 = (o3d ** 2).sum(axis=2)
    var = (s2 - s1 ** 2 / OUT) / (OUT - 1)
    ml = np.float32(var.sum() / B)
    return outs, rl, sl, ml
